# revision 8
# baseline (speedup 1.0000x reference)
"""Trainium2 Bass kernel for nn_Capsule3D (capsule conv + routing softmax + squash).

Strategy (data-parallel over batch, 2 samples per core, 8 cores):
  Per sample b, on device:
    - mini-conv of ubar = sum_i x[b,:,:,i,:] gives t[j,l] (routing sum over input capsules)
    - main conv per input capsule i (weights-stationary K=72 matmul; im2col rows
      are host-prepared shifted views) -> u_hat in PSUM -> evicted to SBUF bf16
      in layout U[(c,l)=128 partitions, (i,pos) free]
    - z_i = sum_l U_i * t  : DVE product + PE "Lrep" matmul (block-diag ones) that
      reduces over l *and* replicates the result over l partitions
    - e = exp(z/sqrt(L)) on ScalarE directly from PSUM (accum_out gives the
      softmax denominator column-sums for free)
    - s = sum_i U_i * e_i / S_i : DVE scalar_tensor_tensor + PE identity-matmul
      accumulation into PSUM, plus t*b_route term
    - squash: norm over l via Lrep matmul on v^2, then (1-exp(-r))/r scaling
  Host side does only layout transforms (transpose/replication/casts) and
  sharding; all math runs on the NeuronCores.
"""

import math

import numpy as np

# ---------------- problem constants (hardcoded per harness contract) ----------
B, H, W, IC, IL = 16, 32, 32, 32, 8
KH = KW = 3
CL = 128
L = 8
C = CL // L            # 16
OH = OW = 30
POS = OH * OW          # 900
HW = H * W             # 1024
K9 = KH * KW * IL      # 72
NCORES = 8
BLOC = B // NCORES     # 2
EPS = 1e-7
RSQRT_L = 1.0 / math.sqrt(float(L))
SHIFTS = [32 * ky + kx for ky in range(KH) for kx in range(KW)]

_CACHE = {}


def _build_nc():
    import concourse.bass as bass
    import concourse.tile as tile
    from concourse import bacc, mybir

    f32 = mybir.dt.float32
    bf16 = mybir.dt.bfloat16
    AF = mybir.ActivationFunctionType
    OP = mybir.AluOpType

    nc = bacc.Bacc()

    xt9_d = nc.dram_tensor("xt9", [BLOC, IC, K9, HW], bf16, kind="ExternalInput")
    xnat_d = nc.dram_tensor("xnat", [BLOC, HW, IC * IL], f32, kind="ExternalInput")
    w72_d = nc.dram_tensor("w72", [K9, CL], bf16, kind="ExternalInput")
    lrep_d = nc.dram_tensor("lrep", [128, 128], bf16, kind="ExternalInput")
    i128_d = nc.dram_tensor("i128", [128, 128], bf16, kind="ExternalInput")
    br_d = nc.dram_tensor("br_cl", [128, POS], f32, kind="ExternalInput")
    y_d = nc.dram_tensor("y", [BLOC, 128, POS], f32, kind="ExternalOutput")

    with tile.TileContext(nc) as tc:
        with (
            tc.tile_pool(name="const", bufs=1) as constp,
            tc.tile_pool(name="xnat", bufs=2) as xnatp,
            tc.tile_pool(name="ub", bufs=2) as ubp,
            tc.tile_pool(name="ubar", bufs=1) as ubarp,
            tc.tile_pool(name="xt9", bufs=3) as xt9p,
            tc.tile_pool(name="bigU", bufs=1) as bigUp,
            tc.tile_pool(name="bigE", bufs=1) as bigEp,
            tc.tile_pool(name="tt", bufs=1) as ttp,
            tc.tile_pool(name="q", bufs=3) as qp,
            tc.tile_pool(name="sm", bufs=2) as smp,
            tc.tile_pool(name="sq", bufs=1) as sqp,
            tc.tile_pool(name="dram", bufs=2, space="DRAM") as dramp,
            tc.tile_pool(name="pu", bufs=2, space="PSUM") as pup,
            tc.tile_pool(name="pz", bufs=2, space="PSUM") as pzp,
        ):
            # ---- constants (loaded once) ----
            w72s = constp.tile([K9, CL], bf16)
            nc.sync.dma_start(out=w72s, in_=w72_d[:, :])
            lreps = constp.tile([128, 128], bf16)
            nc.sync.dma_start(out=lreps, in_=lrep_d[:, :])
            i128s = constp.tile([128, 128], bf16)
            nc.sync.dma_start(out=i128s, in_=i128_d[:, :])
            brs = constp.tile([128, POS], f32)
            nc.sync.dma_start(out=brs, in_=br_d[:, :])
            ones_f = constp.tile([128, 1], f32)
            nc.vector.memset(ones_f, 1.0)
            eps_t = constp.tile([128, 1], f32)
            nc.vector.memset(eps_t, EPS)

            for b in range(BLOC):
                # ---------- ubar path: ubar[il, hw] = sum_i x[b,:,:,i,il] ----------
                ubarT = ubarp.tile([IL, HW], bf16, tag="ubarT")
                xn = xnatp.tile([128, HW // 128, IC * IL], f32, tag="xn")
                nc.sync.dma_start(
                    out=xn, in_=xnat_d[b].rearrange("(t p) f -> p t f", p=128)
                )
                for hwt in range(HW // 128):
                    ub_f = ubp.tile([128, IL], f32, tag="ubf")
                    nc.vector.reduce_sum(
                        out=ub_f,
                        in_=xn[:, hwt, :].rearrange("p (i l) -> p l i", l=IL),
                        axis=mybir.AxisListType.X,
                    )
                    ub_b = ubp.tile([128, IL], bf16, tag="ubb")
                    nc.vector.tensor_copy(out=ub_b, in_=ub_f)
                    ps_tr = pup.tile([IL, 128], bf16, tag="pu")
                    nc.tensor.transpose(ps_tr, ub_b, i128s)
                    nc.scalar.copy(
                        out=ubarT[:, hwt * 128 : (hwt + 1) * 128], in_=ps_tr
                    )
                ubar9 = ubarp.tile([K9, HW], bf16, tag="ubar9")
                for g, s in enumerate(SHIFTS):
                    nc.sync.dma_start(
                        out=ubar9[g * IL : (g + 1) * IL, 0 : HW - s],
                        in_=ubarT[:, s:HW],
                    )

                # ---------- t mini-conv ----------
                psum_t = pup.tile([128, 2, 512], f32, tag="pu")
                ub_v = ubar9.rearrange("p (h w) -> p h w", w=W)
                for h in range(2):
                    nc.tensor.matmul(
                        psum_t[:, h, 0:450],
                        w72s,
                        ub_v[:, 15 * h : 15 * h + 15, 0:OW],
                        start=True,
                        stop=True,
                    )
                t_bf = ttp.tile([128, 2, 450], bf16, tag="tbf")
                nc.scalar.copy(out=t_bf, in_=psum_t[:, :, 0:450])
                t2_f = ttp.tile([128, 2, 450], f32, tag="t2")
                nc.vector.tensor_mul(
                    out=t2_f,
                    in0=psum_t[:, :, 0:450],
                    in1=brs.rearrange("p (h n) -> p h n", h=2),
                )

                # ---------- main conv + eviction (per input capsule i) ----------
                U_A = bigUp.tile([128, IC, POS], bf16, tag="U")
                for i in range(IC):
                    xt9 = xt9p.tile([K9, HW], bf16, tag="xt9")
                    nc.sync.dma_start(out=xt9, in_=xt9_d[b, i])
                    pu = pup.tile([128, 2, 512], f32, tag="pu")
                    xv = xt9.rearrange("p (h w) -> p h w", w=W)
                    for h in range(2):
                        nc.tensor.matmul(
                            pu[:, h, 0:450],
                            w72s,
                            xv[:, 15 * h : 15 * h + 15, 0:OW],
                            start=True,
                            stop=True,
                        )
                    Uv = U_A[:, i, :].rearrange("p (h n) -> p h n", h=2)
                    if i % 3 == 2:
                        nc.vector.tensor_copy(out=Uv, in_=pu[:, :, 0:450])
                    else:
                        nc.scalar.copy(out=Uv, in_=pu[:, :, 0:450])

                # ---------- z phase: product, l-reduce(+replicate), exp ----------
                e_rep = bigEp.tile([128, IC, POS], bf16, tag="E")
                colsum = smp.tile([128, IC], f32, tag="colsum")
                for i in range(IC):
                    q = qp.tile([128, POS], bf16, tag="q")
                    nc.vector.tensor_mul(
                        out=q,
                        in0=U_A[:, i, :],
                        in1=t_bf.rearrange("p h n -> p (h n)"),
                    )
                    pz = pzp.tile([128, 2, 512], f32, tag="pz")
                    qv = q.rearrange("p (h n) -> p h n", h=2)
                    for h in range(2):
                        nc.tensor.matmul(
                            pz[:, h, 0:450], lreps, qv[:, h, :], start=True, stop=True
                        )
                    nc.scalar.activation(
                        out=e_rep[:, i, :].rearrange("p (h n) -> p h n", h=2),
                        in_=pz[:, :, 0:450],
                        func=AF.Exp,
                        scale=RSQRT_L,
                        accum_out=colsum[:, i : i + 1],
                    )

                # ---------- softmax denominator ----------
                psum_S = pup.tile([1, IC], f32, tag="pu")
                nc.tensor.matmul(psum_S, ones_f, colsum, start=True, stop=True)
                srow = smp.tile([1, IC], f32, tag="srow")
                nc.vector.tensor_copy(out=srow, in_=psum_S)
                sinv = smp.tile([1, IC], f32, tag="sinv")
                nc.vector.reciprocal(out=sinv, in_=srow)
                nc.scalar.mul(out=sinv, in_=sinv, mul=float(L))
                dsc = dramp.tile([1, IC], f32, tag="dsc")
                nc.sync.dma_start(out=dsc, in_=sinv)
                sinv_tab = smp.tile([128, IC], f32, tag="stab")
                dv = dsc[0, :]
                bcast = bass.AP(
                    tensor=dv.tensor, offset=dv.offset, ap=[[0, 128]] + list(dv.ap)
                )
                nc.sync.dma_start(out=sinv_tab, in_=bcast)

                # ---------- s phase: weighted sum over i into PSUM ----------
                psum_s = pzp.tile([128, 2, 512], f32, tag="pz")
                for i in range(IC):
                    q2 = qp.tile([128, POS], bf16, tag="q")
                    nc.vector.scalar_tensor_tensor(
                        out=q2,
                        in0=U_A[:, i, :],
                        scalar=sinv_tab[:, i : i + 1],
                        in1=e_rep[:, i, :],
                        op0=OP.mult,
                        op1=OP.mult,
                    )
                    q2v = q2.rearrange("p (h n) -> p h n", h=2)
                    for h in range(2):
                        nc.tensor.matmul(
                            psum_s[:, h, 0:450],
                            i128s,
                            q2v[:, h, :],
                            start=(i == 0),
                            stop=(i == IC - 1),
                        )

                # ---------- squash ----------
                v_sb = sqp.tile([128, 2, 450], f32, tag="vsb")
                nc.vector.tensor_add(out=v_sb, in0=psum_s[:, :, 0:450], in1=t2_f)
                sq_bf = sqp.tile([128, 2, 450], bf16, tag="sqbf")
                nc.scalar.activation(out=sq_bf, in_=v_sb, func=AF.Square)
                pn = pup.tile([128, 2, 512], f32, tag="pu")
                for h in range(2):
                    nc.tensor.matmul(
                        pn[:, h, 0:450], lreps, sq_bf[:, h, :], start=True, stop=True
                    )
                rsb = sqp.tile([128, 2, 450], f32, tag="rsb")
                nc.scalar.activation(
                    out=rsb, in_=pn[:, :, 0:450], func=AF.Sqrt, bias=eps_t
                )
                rinv = sqp.tile([128, 2, 450], f32, tag="rinv")
                nc.vector.reciprocal(out=rinv, in_=rsb)
                g_t = sqp.tile([128, 2, 450], f32, tag="gt")
                nc.scalar.activation(out=g_t, in_=rsb, func=AF.Exp, scale=-1.0)
                nc.vector.tensor_scalar(
                    out=g_t, in0=g_t, scalar1=-1.0, scalar2=1.0, op0=OP.mult, op1=OP.add
                )
                a_t = sqp.tile([128, 2, 450], f32, tag="at")
                nc.vector.tensor_mul(out=a_t, in0=v_sb, in1=rinv)
                o_t = sqp.tile([128, 2, 450], f32, tag="ot")
                nc.vector.tensor_mul(out=o_t, in0=a_t, in1=g_t)
                nc.sync.dma_start(
                    out=y_d[b].rearrange("p (h n) -> p h n", h=2), in_=o_t
                )

    nc.finalize()
    return nc


def _prep_host(x, w, b_route):
    import ml_dtypes

    bf = ml_dtypes.bfloat16
    x = np.ascontiguousarray(np.asarray(x, dtype=np.float32))
    w = np.asarray(w, dtype=np.float32)
    b_route = np.asarray(b_route, dtype=np.float32)

    # xt[b, i, il, hw]
    xt = np.ascontiguousarray(x.transpose(0, 3, 4, 1, 2)).reshape(B, IC, IL, HW)
    xt9 = np.zeros((B, IC, K9, HW), dtype=bf)
    xtb = xt.astype(bf)
    for g, s in enumerate(SHIFTS):
        if s == 0:
            xt9[:, :, g * IL : (g + 1) * IL, :] = xtb
        else:
            xt9[:, :, g * IL : (g + 1) * IL, : HW - s] = xtb[:, :, :, s:]

    xnat = x.reshape(B, HW, IC * IL)

    # W72[(ky,kx,il), cl]
    w72 = np.ascontiguousarray(
        w[:, :, :, 0, :].transpose(1, 2, 0, 3).reshape(K9, CL)
    ).astype(bf)
    lrep = np.kron(np.eye(C, dtype=np.float32), np.ones((L, L), np.float32)).astype(bf)
    i128 = np.eye(128, dtype=np.float32).astype(bf)
    # br_cl[(c*8+l), pos] = b_route[pos*16+c, l]
    br_cl = np.ascontiguousarray(
        b_route.reshape(POS, C, L).transpose(1, 2, 0).reshape(128, POS)
    ).astype(np.float32)
    return xt9, xnat, w72, lrep, i128, br_cl


def kernel(x, w, b_route, stride):
    assert int(stride) == 1
    xt9, xnat, w72, lrep, i128, br_cl = _prep_host(x, w, b_route)

    if "nc" not in _CACHE:
        _CACHE["nc"] = _build_nc()
    nc = _CACHE["nc"]

    from concourse.bass_utils import run_bass_kernel_spmd

    in_maps = []
    for c in range(NCORES):
        sl = slice(c * BLOC, (c + 1) * BLOC)
        in_maps.append(
            {
                "xt9": np.ascontiguousarray(xt9[sl]),
                "xnat": np.ascontiguousarray(xnat[sl]),
                "w72": w72,
                "lrep": lrep,
                "i128": i128,
                "br_cl": br_cl,
            }
        )

    res = run_bass_kernel_spmd(nc, in_maps, core_ids=list(range(NCORES)))

    y = np.empty((B, OH, OW, C, L), dtype=np.float32)
    for c in range(NCORES):
        yd = res.results[c]["y"]  # [BLOC, 128, 900]
        y[c * BLOC : (c + 1) * BLOC] = (
            yd.reshape(BLOC, C, L, POS).transpose(0, 3, 1, 2).reshape(
                BLOC, OH, OW, C, L
            )
        )
    return y


# revision 18
# speedup vs baseline: 1.0107x; 1.0107x over previous
"""Trainium2 Bass kernel for nn_Capsule3D (capsule conv + routing softmax + squash).

Strategy (data-parallel over batch, 2 samples per core, 8 cores):
  Per sample b, on device:
    - mini-conv of ubar = sum_i x[b,:,:,i,:] gives t[j,l] (routing sum over input capsules)
    - main conv per input capsule i (weights-stationary K=72 matmul; im2col rows
      are host-prepared shifted views) -> u_hat in PSUM -> evicted to SBUF bf16
      in layout U[(c,l)=128 partitions, (i,pos) free]
    - z_i = sum_l U_i * t  : DVE product + PE "Lrep" matmul (block-diag ones) that
      reduces over l *and* replicates the result over l partitions
    - e = exp(z/sqrt(L)) on ScalarE directly from PSUM (accum_out gives the
      softmax denominator column-sums for free)
    - s = sum_i U_i * e_i / S_i : DVE scalar_tensor_tensor + PE identity-matmul
      accumulation into PSUM, plus t*b_route term
    - squash: norm over l via Lrep matmul on v^2, then (1-exp(-r))/r scaling
  Host side does only layout transforms (transpose/replication/casts) and
  sharding; all math runs on the NeuronCores.
"""

import math

import numpy as np

# ---------------- problem constants (hardcoded per harness contract) ----------
B, H, W, IC, IL = 16, 32, 32, 32, 8
KH = KW = 3
CL = 128
L = 8
C = CL // L            # 16
OH = OW = 30
POS = OH * OW          # 900
HW = H * W             # 1024
K9 = KH * KW * IL      # 72
NCORES = 8
BLOC = B // NCORES     # 2
EPS = 1e-7
RSQRT_L = 1.0 / math.sqrt(float(L))
SHIFTS = [32 * ky + kx for ky in range(KH) for kx in range(KW)]

_CACHE = {}


def _build_nc():
    import concourse.bass as bass
    import concourse.tile as tile
    from concourse import bacc, mybir

    f32 = mybir.dt.float32
    bf16 = mybir.dt.bfloat16
    AF = mybir.ActivationFunctionType
    OP = mybir.AluOpType

    nc = bacc.Bacc()

    xt9_d = nc.dram_tensor("xt9", [BLOC, IC, K9, HW], bf16, kind="ExternalInput")
    xnat_d = nc.dram_tensor("xnat", [BLOC, HW, IC * IL], f32, kind="ExternalInput")
    w72_d = nc.dram_tensor("w72", [K9, CL], bf16, kind="ExternalInput")
    lrep_d = nc.dram_tensor("lrep", [128, 128], bf16, kind="ExternalInput")
    i128_d = nc.dram_tensor("i128", [128, 128], bf16, kind="ExternalInput")
    br_d = nc.dram_tensor("br_cl", [128, POS], f32, kind="ExternalInput")
    y_d = nc.dram_tensor("y", [BLOC, 128, POS], f32, kind="ExternalOutput")

    HP = 450  # half of the 900 output positions

    with tile.TileContext(nc) as tc:
        with (
            tc.tile_pool(name="const", bufs=1) as constp,
            tc.tile_pool(name="xnat", bufs=2) as xnatp,
            tc.tile_pool(name="ub", bufs=2) as ubp,
            tc.tile_pool(name="ubar", bufs=1) as ubarp,
            tc.tile_pool(name="xt9", bufs=3) as xt9p,
            tc.tile_pool(name="utmp", bufs=4) as utmpp,
            tc.tile_pool(name="etmp", bufs=3) as etmpp,
            tc.tile_pool(name="q2s", bufs=IC) as q2p,
            tc.tile_pool(name="tt", bufs=2) as ttp,
            tc.tile_pool(name="q", bufs=6) as qp,
            tc.tile_pool(name="sip", bufs=IC) as sip,
            tc.tile_pool(name="sm", bufs=2) as smp,
            tc.tile_pool(name="sq", bufs=2) as sqp,
            tc.tile_pool(name="dram", bufs=2, space="DRAM") as dramp,
            tc.tile_pool(name="pu", bufs=3, space="PSUM") as pup,
            tc.tile_pool(name="pz", bufs=2, space="PSUM") as pzp,
            tc.tile_pool(name="lt", bufs=1, space="PSUM") as ltp,
        ):
            # ---- constants (loaded once) ----
            w72s = constp.tile([K9, CL], bf16)
            nc.sync.dma_start(out=w72s, in_=w72_d[:, :])
            lreps = constp.tile([128, 128], bf16)
            nc.sync.dma_start(out=lreps, in_=lrep_d[:, :])
            i128s = constp.tile([128, 128], bf16)
            nc.sync.dma_start(out=i128s, in_=i128_d[:, :])
            brs = constp.tile([128, POS], f32)
            nc.sync.dma_start(out=brs, in_=br_d[:, :])
            eps_t = constp.tile([128, 1], f32)
            nc.vector.memset(eps_t, EPS)

            for b in range(BLOC):
                # ---------- ubar path ----------
                ubarT = ubarp.tile([IL, HW], bf16, tag="ubarT")
                xn = xnatp.tile([128, HW // 128, IC * IL], f32, tag="xn")
                nc.sync.dma_start(
                    out=xn, in_=xnat_d[b].rearrange("(t p) f -> p t f", p=128)
                )
                for hwt in range(HW // 128):
                    ub_f = ubp.tile([128, IL], f32, tag="ubf")
                    nc.vector.reduce_sum(
                        out=ub_f,
                        in_=xn[:, hwt, :].rearrange("p (i l) -> p l i", l=IL),
                        axis=mybir.AxisListType.X,
                    )
                    ub_b = ubp.tile([128, IL], bf16, tag="ubb")
                    nc.vector.tensor_copy(out=ub_b, in_=ub_f)
                    ps_tr = pup.tile([IL, 128], bf16, tag="pu")
                    nc.tensor.transpose(ps_tr, ub_b, i128s)
                    nc.vector.tensor_copy(
                        out=ubarT[:, hwt * 128 : (hwt + 1) * 128], in_=ps_tr
                    )
                ubar9 = ubarp.tile([K9, HW], bf16, tag="ubar9")
                for g, s in enumerate(SHIFTS):
                    nc.sync.dma_start(
                        out=ubar9[g * IL : (g + 1) * IL, 0 : HW - s],
                        in_=ubarT[:, s:HW],
                    )

                # ---------- t mini-conv (per half) ----------
                ub_v = ubar9.rearrange("p (h w) -> p h w", w=W)
                t_bf = ttp.tile([128, 2, HP], bf16, tag="tbf")
                t2_f = ttp.tile([128, 2, HP], f32, tag="t2")
                brv = brs.rearrange("p (h n) -> p h n", h=2)
                for h in range(2):
                    psum_t = pup.tile([128, 512], f32, tag="pu")
                    nc.tensor.matmul(
                        psum_t[:, 0:HP],
                        w72s,
                        ub_v[:, 15 * h : 15 * h + 15, 0:OW],
                        start=True,
                        stop=True,
                    )
                    nc.scalar.copy(out=t_bf[:, h, :], in_=psum_t[:, 0:HP])
                    nc.vector.tensor_mul(
                        out=t2_f[:, h, :], in0=psum_t[:, 0:HP], in1=brv[:, h, :]
                    )

                # ---------- main loop: conv, evict, z, l-reduce, exp, q2 ----------
                q2_tiles = []
                colsum = smp.tile([128, IC], f32, tag="colsum")
                for i in range(IC):
                    xt9 = xt9p.tile([K9, HW], bf16, tag="xt9")
                    nc.sync.dma_start(out=xt9, in_=xt9_d[b, i])
                    xv = xt9.rearrange("p (h w) -> p h w", w=W)
                    U_i = utmpp.tile([128, 2, HP], bf16, tag="ut")
                    e_i = etmpp.tile([128, 2, HP], bf16, tag="et")
                    pz = pzp.tile([128, 2, 512], f32, tag="pz")
                    for h in range(2):
                        pu = pup.tile([128, 512], f32, tag="pu")
                        nc.tensor.matmul(
                            pu[:, 0:HP],
                            w72s,
                            xv[:, 15 * h : 15 * h + 15, 0:OW],
                            start=True,
                            stop=True,
                        )
                        if (2 * i + h) * 9 % 20 < 9:
                            nc.vector.tensor_copy(out=U_i[:, h, :], in_=pu[:, 0:HP])
                        else:
                            nc.scalar.copy(out=U_i[:, h, :], in_=pu[:, 0:HP])
                    q = qp.tile([128, 2, HP], bf16, tag="q")
                    nc.vector.tensor_mul(out=q, in0=U_i, in1=t_bf)
                    for h in range(2):
                        nc.tensor.matmul(
                            pz[:, h, 0:HP], lreps, q[:, h, :], start=True, stop=True
                        )
                    nc.scalar.activation(
                        out=e_i,
                        in_=pz[:, :, 0:HP],
                        func=AF.Exp,
                        scale=RSQRT_L,
                        accum_out=colsum[:, i : i + 1],
                    )
                    q2_i = q2p.tile([128, 2, HP], bf16, tag="q2", name=f"q2_{i}")
                    q2_tiles.append(q2_i)
                    nc.vector.tensor_mul(out=q2_i, in0=U_i, in1=e_i)

                # ---------- softmax denominator: allreduce over partitions ----------
                from concourse import bass_isa

                s_all = smp.tile([128, IC], f32, tag="sall")
                nc.gpsimd.partition_all_reduce(
                    s_all, colsum, 128, bass_isa.ReduceOp.add
                )
                sinv_tab = smp.tile([128, IC], f32, tag="stab")
                nc.vector.reciprocal(out=sinv_tab, in_=s_all)

                # ---------- s phase + squash, one half at a time ----------
                si_tiles = []
                for i in range(IC):
                    si = sip.tile([128, 128], bf16, tag="si", name=f"si{i}")
                    nc.vector.tensor_scalar(
                        out=si,
                        in0=i128s,
                        scalar1=sinv_tab[:, i : i + 1],
                        scalar2=float(L),
                        op0=OP.mult,
                        op1=OP.mult,
                    )
                    si_tiles.append(si)
                o_t = sqp.tile([128, 2, HP], f32, tag="ot")
                for h in range(2):
                    psum_s = ltp.tile([128, 512], f32, tag="lt")
                    for i in range(IC):
                        nc.tensor.matmul(
                            psum_s[:, 0:HP],
                            si_tiles[i],
                            q2_tiles[i][:, h, :],
                            start=(i == 0),
                            stop=(i == IC - 1),
                        )
                    v_sb = sqp.tile([128, HP], f32, tag="vsb")
                    nc.vector.tensor_add(
                        out=v_sb, in0=psum_s[:, 0:HP], in1=t2_f[:, h, :]
                    )
                    sq_bf = sqp.tile([128, HP], bf16, tag="sqbf")
                    nc.scalar.activation(out=sq_bf, in_=v_sb, func=AF.Square)
                    pn = ltp.tile([128, 512], f32, tag="lt")
                    nc.tensor.matmul(
                        pn[:, 0:HP], lreps, sq_bf, start=True, stop=True
                    )
                    rsb = sqp.tile([128, HP], f32, tag="rsb")
                    nc.scalar.activation(
                        out=rsb, in_=pn[:, 0:HP], func=AF.Sqrt, bias=eps_t
                    )
                    rinv = sqp.tile([128, HP], f32, tag="rinv")
                    nc.vector.reciprocal(out=rinv, in_=rsb)
                    g_t = sqp.tile([128, HP], f32, tag="gt")
                    nc.scalar.activation(out=g_t, in_=rsb, func=AF.Exp, scale=-1.0)
                    nc.vector.tensor_scalar(
                        out=g_t,
                        in0=g_t,
                        scalar1=-1.0,
                        scalar2=1.0,
                        op0=OP.mult,
                        op1=OP.add,
                    )
                    a_t = sqp.tile([128, HP], f32, tag="at")
                    nc.vector.tensor_mul(out=a_t, in0=v_sb, in1=rinv)
                    nc.vector.tensor_mul(out=o_t[:, h, :], in0=a_t, in1=g_t)
                nc.gpsimd.dma_start(
                    out=y_d[b].rearrange("p (h n) -> p h n", h=2), in_=o_t
                )

    nc.finalize()
    return nc


def _prep_host(x, w, b_route):
    import ml_dtypes

    bf = ml_dtypes.bfloat16
    x = np.ascontiguousarray(np.asarray(x, dtype=np.float32))
    w = np.asarray(w, dtype=np.float32)
    b_route = np.asarray(b_route, dtype=np.float32)

    # xt[b, i, il, hw]
    xt = np.ascontiguousarray(x.transpose(0, 3, 4, 1, 2)).reshape(B, IC, IL, HW)
    xt9 = np.zeros((B, IC, K9, HW), dtype=bf)
    xtb = xt.astype(bf)
    for g, s in enumerate(SHIFTS):
        if s == 0:
            xt9[:, :, g * IL : (g + 1) * IL, :] = xtb
        else:
            xt9[:, :, g * IL : (g + 1) * IL, : HW - s] = xtb[:, :, :, s:]

    xnat = x.reshape(B, HW, IC * IL)

    # W72[(ky,kx,il), cl]
    w72 = np.ascontiguousarray(
        w[:, :, :, 0, :].transpose(1, 2, 0, 3).reshape(K9, CL)
    ).astype(bf)
    lrep = np.kron(np.eye(C, dtype=np.float32), np.ones((L, L), np.float32)).astype(bf)
    i128 = np.eye(128, dtype=np.float32).astype(bf)
    # br_cl[(c*8+l), pos] = b_route[pos*16+c, l]
    br_cl = np.ascontiguousarray(
        b_route.reshape(POS, C, L).transpose(1, 2, 0).reshape(128, POS)
    ).astype(np.float32)
    return xt9, xnat, w72, lrep, i128, br_cl


def kernel(x, w, b_route, stride):
    assert int(stride) == 1
    xt9, xnat, w72, lrep, i128, br_cl = _prep_host(x, w, b_route)

    if "nc" not in _CACHE:
        _CACHE["nc"] = _build_nc()
    nc = _CACHE["nc"]

    from concourse.bass_utils import run_bass_kernel_spmd

    in_maps = []
    for c in range(NCORES):
        sl = slice(c * BLOC, (c + 1) * BLOC)
        in_maps.append(
            {
                "xt9": np.ascontiguousarray(xt9[sl]),
                "xnat": np.ascontiguousarray(xnat[sl]),
                "w72": w72,
                "lrep": lrep,
                "i128": i128,
                "br_cl": br_cl,
            }
        )

    res = run_bass_kernel_spmd(nc, in_maps, core_ids=list(range(NCORES)))

    y = np.empty((B, OH, OW, C, L), dtype=np.float32)
    for c in range(NCORES):
        yd = res.results[c]["y"]  # [BLOC, 128, 900]
        y[c * BLOC : (c + 1) * BLOC] = (
            yd.reshape(BLOC, C, L, POS).transpose(0, 3, 1, 2).reshape(
                BLOC, OH, OW, C, L
            )
        )
    return y


# revision 23
# speedup vs baseline: 7091.7486x; 7016.8075x over previous
"""Trainium2 Bass kernel for nn_Capsule3D (capsule conv + routing softmax + squash).

Strategy (data-parallel over batch, 2 samples per core, 8 cores):
  Per sample b, on device:
    - mini-conv of ubar = sum_i x[b,:,:,i,:] gives t[j,l] (routing sum over input capsules)
    - main conv per input capsule i (weights-stationary K=72 matmul; im2col rows
      are host-prepared shifted views) -> u_hat in PSUM -> evicted to SBUF bf16
      in layout U[(c,l)=128 partitions, (i,pos) free]
    - z_i = sum_l U_i * t  : DVE product + PE "Lrep" matmul (block-diag ones) that
      reduces over l *and* replicates the result over l partitions
    - e = exp(z/sqrt(L)) on ScalarE directly from PSUM (accum_out gives the
      softmax denominator column-sums for free)
    - s = sum_i U_i * e_i / S_i : DVE scalar_tensor_tensor + PE identity-matmul
      accumulation into PSUM, plus t*b_route term
    - squash: norm over l via Lrep matmul on v^2, then (1-exp(-r))/r scaling
  Host side does only layout transforms (transpose/replication/casts) and
  sharding; all math runs on the NeuronCores.
"""

import math

import numpy as np

# ---------------- problem constants (hardcoded per harness contract) ----------
B, H, W, IC, IL = 16, 32, 32, 32, 8
KH = KW = 3
CL = 128
L = 8
C = CL // L            # 16
OH = OW = 30
POS = OH * OW          # 900
HW = H * W             # 1024
K9 = KH * KW * IL      # 72
NCORES = 8
BLOC = B // NCORES     # 2
EPS = 1e-7
RSQRT_L = 1.0 / math.sqrt(float(L))
SHIFTS = [32 * ky + kx for ky in range(KH) for kx in range(KW)]

_CACHE = {}


def _build_nc():
    import concourse.bass as bass
    import concourse.tile as tile
    from concourse import bacc, mybir

    f32 = mybir.dt.float32
    bf16 = mybir.dt.bfloat16
    AF = mybir.ActivationFunctionType
    OP = mybir.AluOpType

    nc = bacc.Bacc()

    xt9_d = nc.dram_tensor("xt9", [BLOC, IC, K9, HW], bf16, kind="ExternalInput")
    xnat_d = nc.dram_tensor("xnat", [BLOC, HW, IC * IL], f32, kind="ExternalInput")
    w72_d = nc.dram_tensor("w72", [K9, CL], bf16, kind="ExternalInput")
    lrep_d = nc.dram_tensor("lrep", [128, 128], bf16, kind="ExternalInput")
    i128_d = nc.dram_tensor("i128", [128, 128], bf16, kind="ExternalInput")
    br_d = nc.dram_tensor("br_cl", [128, POS], f32, kind="ExternalInput")
    y_d = nc.dram_tensor("y", [BLOC, 128, POS], f32, kind="ExternalOutput")

    HP = 450  # half of the 900 output positions

    with tile.TileContext(nc) as tc:
        with (
            tc.tile_pool(name="const", bufs=1) as constp,
            tc.tile_pool(name="xnat", bufs=2) as xnatp,
            tc.tile_pool(name="ub", bufs=2) as ubp,
            tc.tile_pool(name="ubar", bufs=1) as ubarp,
            tc.tile_pool(name="xt9", bufs=4) as xt9p,
            tc.tile_pool(name="utmp", bufs=5) as utmpp,
            tc.tile_pool(name="etmp", bufs=4) as etmpp,
            tc.tile_pool(name="q2s", bufs=IC) as q2p,
            tc.tile_pool(name="tt", bufs=2) as ttp,
            tc.tile_pool(name="q", bufs=6) as qp,
            tc.tile_pool(name="sip", bufs=IC) as sip,
            tc.tile_pool(name="sm", bufs=2) as smp,
            tc.tile_pool(name="sq", bufs=2) as sqp,
            tc.tile_pool(name="dram", bufs=2, space="DRAM") as dramp,
            tc.tile_pool(name="pu", bufs=3, space="PSUM") as pup,
            tc.tile_pool(name="pz", bufs=2, space="PSUM") as pzp,
            tc.tile_pool(name="lt", bufs=1, space="PSUM") as ltp,
        ):
            # ---- constants (loaded once) ----
            w72s = constp.tile([K9, CL], bf16)
            nc.sync.dma_start(out=w72s, in_=w72_d[:, :])
            lreps = constp.tile([128, 128], bf16)
            nc.sync.dma_start(out=lreps, in_=lrep_d[:, :])
            i128s = constp.tile([128, 128], bf16)
            nc.sync.dma_start(out=i128s, in_=i128_d[:, :])
            brs = constp.tile([128, POS], f32)
            nc.sync.dma_start(out=brs, in_=br_d[:, :])
            eps_t = constp.tile([128, 1], f32)
            nc.vector.memset(eps_t, EPS)

            for b in range(BLOC):
                # ---------- ubar path ----------
                ubarT = ubarp.tile([IL, HW], bf16, tag="ubarT")
                xn = xnatp.tile([128, HW // 128, IC * IL], f32, tag="xn")
                nc.sync.dma_start(
                    out=xn, in_=xnat_d[b].rearrange("(t p) f -> p t f", p=128)
                )
                for hwt in range(HW // 128):
                    ub_f = ubp.tile([128, IL], f32, tag="ubf")
                    nc.vector.reduce_sum(
                        out=ub_f,
                        in_=xn[:, hwt, :].rearrange("p (i l) -> p l i", l=IL),
                        axis=mybir.AxisListType.X,
                    )
                    ub_b = ubp.tile([128, IL], bf16, tag="ubb")
                    nc.vector.tensor_copy(out=ub_b, in_=ub_f)
                    ps_tr = pup.tile([IL, 128], bf16, tag="pu")
                    nc.tensor.transpose(ps_tr, ub_b, i128s)
                    nc.vector.tensor_copy(
                        out=ubarT[:, hwt * 128 : (hwt + 1) * 128], in_=ps_tr
                    )
                ubar9 = ubarp.tile([K9, HW], bf16, tag="ubar9")
                for g, s in enumerate(SHIFTS):
                    nc.sync.dma_start(
                        out=ubar9[g * IL : (g + 1) * IL, 0 : HW - s],
                        in_=ubarT[:, s:HW],
                    )

                # ---------- t mini-conv (per half) ----------
                ub_v = ubar9.rearrange("p (h w) -> p h w", w=W)
                t_bf = ttp.tile([128, 2, HP], bf16, tag="tbf")
                t2_f = ttp.tile([128, 2, HP], f32, tag="t2")
                brv = brs.rearrange("p (h n) -> p h n", h=2)
                for h in range(2):
                    psum_t = pup.tile([128, 512], f32, tag="pu")
                    nc.tensor.matmul(
                        psum_t[:, 0:HP],
                        w72s,
                        ub_v[:, 15 * h : 15 * h + 15, 0:OW],
                        start=True,
                        stop=True,
                    )
                    nc.scalar.copy(out=t_bf[:, h, :], in_=psum_t[:, 0:HP])
                    nc.vector.tensor_mul(
                        out=t2_f[:, h, :], in0=psum_t[:, 0:HP], in1=brv[:, h, :]
                    )

                # ---------- main loop: conv, evict, z, l-reduce, exp, q2 ----------
                q2_tiles = []
                colsum = smp.tile([128, IC], f32, tag="colsum")
                for i in range(IC):
                    xt9 = xt9p.tile([K9, HW], bf16, tag="xt9")
                    nc.sync.dma_start(out=xt9, in_=xt9_d[b, i])
                    xv = xt9.rearrange("p (h w) -> p h w", w=W)
                    U_i = utmpp.tile([128, 2, HP], bf16, tag="ut")
                    e_i = etmpp.tile([128, 2, HP], bf16, tag="et")
                    pz = pzp.tile([128, 2, 512], f32, tag="pz")
                    for h in range(2):
                        pu = pup.tile([128, 512], f32, tag="pu")
                        nc.tensor.matmul(
                            pu[:, 0:HP],
                            w72s,
                            xv[:, 15 * h : 15 * h + 15, 0:OW],
                            start=True,
                            stop=True,
                        )
                        if (2 * i + h) * 9 % 20 < 9:
                            nc.vector.tensor_copy(out=U_i[:, h, :], in_=pu[:, 0:HP])
                        else:
                            nc.scalar.copy(out=U_i[:, h, :], in_=pu[:, 0:HP])
                    q = qp.tile([128, 2, HP], bf16, tag="q")
                    nc.vector.tensor_mul(out=q, in0=U_i, in1=t_bf)
                    for h in range(2):
                        nc.tensor.matmul(
                            pz[:, h, 0:HP], lreps, q[:, h, :], start=True, stop=True
                        )
                    nc.scalar.activation(
                        out=e_i,
                        in_=pz[:, :, 0:HP],
                        func=AF.Exp,
                        scale=RSQRT_L,
                        accum_out=colsum[:, i : i + 1],
                    )
                    q2_i = q2p.tile([128, 2, HP], bf16, tag="q2", name=f"q2_{i}")
                    q2_tiles.append(q2_i)
                    nc.vector.tensor_mul(out=q2_i, in0=U_i, in1=e_i)

                # ---------- softmax denominator: allreduce over partitions ----------
                from concourse import bass_isa

                s_all = smp.tile([128, IC], f32, tag="sall")
                nc.gpsimd.partition_all_reduce(
                    s_all, colsum, 128, bass_isa.ReduceOp.add
                )
                sinv_tab = smp.tile([128, IC], f32, tag="stab")
                nc.vector.reciprocal(out=sinv_tab, in_=s_all)

                # ---------- s phase + squash, one half at a time ----------
                si_tiles = []
                for i in range(IC):
                    si = sip.tile([128, 128], bf16, tag="si", name=f"si{i}")
                    nc.gpsimd.tensor_scalar(
                        out=si,
                        in0=i128s,
                        scalar1=sinv_tab[:, i : i + 1],
                        scalar2=float(L),
                        op0=OP.mult,
                        op1=OP.mult,
                    )
                    si_tiles.append(si)
                o_t = sqp.tile([128, 2, HP], f32, tag="ot")
                for h in range(2):
                    psum_s = ltp.tile([128, 512], f32, tag="lt")
                    for i in range(IC):
                        nc.tensor.matmul(
                            psum_s[:, 0:HP],
                            si_tiles[i],
                            q2_tiles[i][:, h, :],
                            start=(i == 0),
                            stop=(i == IC - 1),
                        )
                    v_sb = sqp.tile([128, HP], f32, tag="vsb")
                    nc.vector.tensor_add(
                        out=v_sb, in0=psum_s[:, 0:HP], in1=t2_f[:, h, :]
                    )
                    sq_bf = sqp.tile([128, HP], bf16, tag="sqbf")
                    nc.scalar.activation(out=sq_bf, in_=v_sb, func=AF.Square)
                    pn = ltp.tile([128, 512], f32, tag="lt")
                    nc.tensor.matmul(
                        pn[:, 0:HP], lreps, sq_bf, start=True, stop=True
                    )
                    rsb = sqp.tile([128, HP], f32, tag="rsb")
                    nc.scalar.activation(
                        out=rsb, in_=pn[:, 0:HP], func=AF.Sqrt, bias=eps_t
                    )
                    rinv = sqp.tile([128, HP], f32, tag="rinv")
                    nc.vector.reciprocal(out=rinv, in_=rsb)
                    g_t = sqp.tile([128, HP], f32, tag="gt")
                    nc.scalar.activation(out=g_t, in_=rsb, func=AF.Exp, scale=-1.0)
                    nc.vector.tensor_scalar(
                        out=g_t,
                        in0=g_t,
                        scalar1=-1.0,
                        scalar2=1.0,
                        op0=OP.mult,
                        op1=OP.add,
                    )
                    a_t = sqp.tile([128, HP], f32, tag="at")
                    nc.vector.tensor_mul(out=a_t, in0=v_sb, in1=rinv)
                    nc.vector.tensor_mul(out=o_t[:, h, :], in0=a_t, in1=g_t)
                nc.gpsimd.dma_start(
                    out=y_d[b].rearrange("p (h n) -> p h n", h=2), in_=o_t
                )

    nc.finalize()
    return nc


def _prep_host(x, w, b_route):
    import ml_dtypes

    bf = ml_dtypes.bfloat16
    x = np.ascontiguousarray(np.asarray(x, dtype=np.float32))
    w = np.asarray(w, dtype=np.float32)
    b_route = np.asarray(b_route, dtype=np.float32)

    # xt[b, i, il, hw]
    xt = np.ascontiguousarray(x.transpose(0, 3, 4, 1, 2)).reshape(B, IC, IL, HW)
    xt9 = np.zeros((B, IC, K9, HW), dtype=bf)
    xtb = xt.astype(bf)
    for g, s in enumerate(SHIFTS):
        if s == 0:
            xt9[:, :, g * IL : (g + 1) * IL, :] = xtb
        else:
            xt9[:, :, g * IL : (g + 1) * IL, : HW - s] = xtb[:, :, :, s:]

    xnat = x.reshape(B, HW, IC * IL)

    # W72[(ky,kx,il), cl]
    w72 = np.ascontiguousarray(
        w[:, :, :, 0, :].transpose(1, 2, 0, 3).reshape(K9, CL)
    ).astype(bf)
    lrep = np.kron(np.eye(C, dtype=np.float32), np.ones((L, L), np.float32)).astype(bf)
    i128 = np.eye(128, dtype=np.float32).astype(bf)
    # br_cl[(c*8+l), pos] = b_route[pos*16+c, l]
    br_cl = np.ascontiguousarray(
        b_route.reshape(POS, C, L).transpose(1, 2, 0).reshape(128, POS)
    ).astype(np.float32)
    return xt9, xnat, w72, lrep, i128, br_cl


def kernel(x, w, b_route, stride):
    assert int(stride) == 1
    xt9, xnat, w72, lrep, i128, br_cl = _prep_host(x, w, b_route)

    if "nc" not in _CACHE:
        _CACHE["nc"] = _build_nc()
    nc = _CACHE["nc"]

    from concourse.bass_utils import run_bass_kernel_spmd

    in_maps = []
    for c in range(NCORES):
        sl = slice(c * BLOC, (c + 1) * BLOC)
        in_maps.append(
            {
                "xt9": np.ascontiguousarray(xt9[sl]),
                "xnat": np.ascontiguousarray(xnat[sl]),
                "w72": w72,
                "lrep": lrep,
                "i128": i128,
                "br_cl": br_cl,
            }
        )

    res = run_bass_kernel_spmd(nc, in_maps, core_ids=list(range(NCORES)))

    y = np.empty((B, OH, OW, C, L), dtype=np.float32)
    for c in range(NCORES):
        yd = res.results[c]["y"]  # [BLOC, 128, 900]
        y[c * BLOC : (c + 1) * BLOC] = (
            yd.reshape(BLOC, C, L, POS).transpose(0, 3, 1, 2).reshape(
                BLOC, OH, OW, C, L
            )
        )
    return y


# revision 29
# speedup vs baseline: 7229.0365x; 1.0194x over previous
"""Trainium2 Bass/Tile kernel for nn_Capsule3D (capsule conv + routing softmax + squash).

Sharding: data-parallel over batch, 2 samples per core x 8 cores. Host side does
only layout transforms (transpose / 9-shift im2col row replication / dtype casts)
and sharding; all math runs on the NeuronCores.

Per sample b, on device (layout: partitions = (c,l) = 128 output channels,
free = output positions pos = 900, per input capsule i = 0..31):
  - t = conv(sum_i x_i) via a mini 72x128 matmul (conv is linear in its input,
    so the routing sum over capsules commutes with the conv)
  - main loop per i: K=72 weights-stationary conv matmul -> PSUM; evict to bf16
    (split ScalarE/VectorE); q = u_hat*t (VectorE bf16 2x); "Lrep" matmul with a
    block-diagonal ones matrix reduces over l AND replicates the result over the
    l partitions; exp((z)/sqrt L) on ScalarE straight from PSUM with accum_out
    giving the softmax denominator column-sums for free; q2 = u_hat*e (VectorE).
  - softmax denominators via gpsimd partition_all_reduce; 1/S_i folded into
    per-i scaled-identity matmuls ("si") built on GpSimd.
  - s = sum_i si^T @ q2_i accumulated in PSUM by TensorE (+ t*b_route term),
    so the s-phase is matmul-only and overlaps the next sample's main loop.
  - squash: norm over l via Lrep matmul on v^2, then v*(1-exp(-r))/r.

The softmax skips the max-subtraction (logits are O(5), safe in fp32 exp).
Intermediates are bf16 (measured end-to-end error ~9e-3 scale-relative absmax
vs the fp32 reference, i.e. ~1e-2 l2-relative, under the 2e-2 gate).
"""

import math

import numpy as np

# ---------------- problem constants (hardcoded per harness contract) ----------
B, H, W, IC, IL = 16, 32, 32, 32, 8
KH = KW = 3
CL = 128
L = 8
C = CL // L            # 16
OH = OW = 30
POS = OH * OW          # 900
HW = H * W             # 1024
K9 = KH * KW * IL      # 72
NCORES = 8
BLOC = B // NCORES     # 2
EPS = 1e-7
RSQRT_L = 1.0 / math.sqrt(float(L))
SHIFTS = [32 * ky + kx for ky in range(KH) for kx in range(KW)]

_CACHE = {}


def _build_nc():
    import concourse.tile as tile
    from concourse import bacc, mybir

    f32 = mybir.dt.float32
    bf16 = mybir.dt.bfloat16
    AF = mybir.ActivationFunctionType
    OP = mybir.AluOpType

    nc = bacc.Bacc()

    xt9_d = nc.dram_tensor("xt9", [BLOC, IC, K9, HW], bf16, kind="ExternalInput")
    xnat_d = nc.dram_tensor("xnat", [BLOC, HW, IC * IL], f32, kind="ExternalInput")
    w72_d = nc.dram_tensor("w72", [K9, CL], bf16, kind="ExternalInput")
    lrep_d = nc.dram_tensor("lrep", [128, 128], bf16, kind="ExternalInput")
    i128_d = nc.dram_tensor("i128", [128, 128], bf16, kind="ExternalInput")
    br_d = nc.dram_tensor("br_cl", [128, POS], f32, kind="ExternalInput")
    y_d = nc.dram_tensor("y", [BLOC, 128, POS], f32, kind="ExternalOutput")

    HP = 450  # half of the 900 output positions

    with tile.TileContext(nc) as tc:
        with (
            tc.tile_pool(name="const", bufs=1) as constp,
            tc.tile_pool(name="xnat", bufs=2) as xnatp,
            tc.tile_pool(name="ub", bufs=2) as ubp,
            tc.tile_pool(name="ubar", bufs=1) as ubarp,
            tc.tile_pool(name="xt9", bufs=4) as xt9p,
            tc.tile_pool(name="utmp", bufs=5) as utmpp,
            tc.tile_pool(name="etmp", bufs=4) as etmpp,
            tc.tile_pool(name="q2s", bufs=IC) as q2p,
            tc.tile_pool(name="tt", bufs=2) as ttp,
            tc.tile_pool(name="q", bufs=6) as qp,
            tc.tile_pool(name="sip", bufs=IC) as sip,
            tc.tile_pool(name="sm", bufs=2) as smp,
            tc.tile_pool(name="sq", bufs=2) as sqp,
            tc.tile_pool(name="pu", bufs=3, space="PSUM") as pup,
            tc.tile_pool(name="pz", bufs=2, space="PSUM") as pzp,
            tc.tile_pool(name="lt", bufs=1, space="PSUM") as ltp,
        ):
            # ---- constants (loaded once) ----
            w72s = constp.tile([K9, CL], bf16)
            nc.sync.dma_start(out=w72s, in_=w72_d[:, :])
            lreps = constp.tile([128, 128], bf16)
            nc.sync.dma_start(out=lreps, in_=lrep_d[:, :])
            i128s = constp.tile([128, 128], bf16)
            nc.sync.dma_start(out=i128s, in_=i128_d[:, :])
            brs = constp.tile([128, POS], f32)
            nc.sync.dma_start(out=brs, in_=br_d[:, :])
            eps_t = constp.tile([128, 1], f32)
            nc.vector.memset(eps_t, EPS)

            for b in range(BLOC):
                # ---------- ubar path ----------
                ubarT = ubarp.tile([IL, HW], bf16, tag="ubarT")
                xn = xnatp.tile([128, HW // 128, IC * IL], f32, tag="xn")
                nc.sync.dma_start(
                    out=xn, in_=xnat_d[b].rearrange("(t p) f -> p t f", p=128)
                )
                for hwt in range(HW // 128):
                    ub_f = ubp.tile([128, IL], f32, tag="ubf")
                    nc.vector.reduce_sum(
                        out=ub_f,
                        in_=xn[:, hwt, :].rearrange("p (i l) -> p l i", l=IL),
                        axis=mybir.AxisListType.X,
                    )
                    ub_b = ubp.tile([128, IL], bf16, tag="ubb")
                    nc.vector.tensor_copy(out=ub_b, in_=ub_f)
                    ps_tr = pup.tile([IL, 128], bf16, tag="pu")
                    nc.tensor.transpose(ps_tr, ub_b, i128s)
                    nc.vector.tensor_copy(
                        out=ubarT[:, hwt * 128 : (hwt + 1) * 128], in_=ps_tr
                    )
                ubar9 = ubarp.tile([K9, HW], bf16, tag="ubar9")
                for g, s in enumerate(SHIFTS):
                    nc.sync.dma_start(
                        out=ubar9[g * IL : (g + 1) * IL, 0 : HW - s],
                        in_=ubarT[:, s:HW],
                    )

                # ---------- t mini-conv (per half) ----------
                ub_v = ubar9.rearrange("p (h w) -> p h w", w=W)
                t_bf = ttp.tile([128, 2, HP], bf16, tag="tbf")
                t2_f = ttp.tile([128, 2, HP], f32, tag="t2")
                brv = brs.rearrange("p (h n) -> p h n", h=2)
                for h in range(2):
                    psum_t = pup.tile([128, 512], f32, tag="pu")
                    nc.tensor.matmul(
                        psum_t[:, 0:HP],
                        w72s,
                        ub_v[:, 15 * h : 15 * h + 15, 0:OW],
                        start=True,
                        stop=True,
                    )
                    nc.scalar.copy(out=t_bf[:, h, :], in_=psum_t[:, 0:HP])
                    nc.vector.tensor_mul(
                        out=t2_f[:, h, :], in0=psum_t[:, 0:HP], in1=brv[:, h, :]
                    )

                # ---------- main loop: conv, evict, z, l-reduce, exp, q2 ----------
                q2_tiles = []
                colsum = smp.tile([128, IC], f32, tag="colsum")
                for i in range(IC):
                    xt9 = xt9p.tile([K9, HW], bf16, tag="xt9")
                    nc.sync.dma_start(out=xt9, in_=xt9_d[b, i])
                    xv = xt9.rearrange("p (h w) -> p h w", w=W)
                    U_i = utmpp.tile([128, 2, HP], bf16, tag="ut")
                    e_i = etmpp.tile([128, 2, HP], bf16, tag="et")
                    pz = pzp.tile([128, 2, 512], f32, tag="pz")
                    for h in range(2):
                        pu = pup.tile([128, 512], f32, tag="pu")
                        nc.tensor.matmul(
                            pu[:, 0:HP],
                            w72s,
                            xv[:, 15 * h : 15 * h + 15, 0:OW],
                            start=True,
                            stop=True,
                        )
                        if (2 * i + h) % 2 == 0:
                            nc.vector.tensor_copy(out=U_i[:, h, :], in_=pu[:, 0:HP])
                        else:
                            nc.scalar.copy(out=U_i[:, h, :], in_=pu[:, 0:HP])
                    q = qp.tile([128, 2, HP], bf16, tag="q")
                    nc.vector.tensor_mul(out=q, in0=U_i, in1=t_bf)
                    for h in range(2):
                        nc.tensor.matmul(
                            pz[:, h, 0:HP], lreps, q[:, h, :], start=True, stop=True
                        )
                    nc.scalar.activation(
                        out=e_i,
                        in_=pz[:, :, 0:HP],
                        func=AF.Exp,
                        scale=RSQRT_L,
                        accum_out=colsum[:, i : i + 1],
                    )
                    q2_i = q2p.tile([128, 2, HP], bf16, tag="q2", name=f"q2_{i}")
                    q2_tiles.append(q2_i)
                    nc.vector.tensor_mul(out=q2_i, in0=U_i, in1=e_i)

                # ---------- softmax denominator: allreduce over partitions ----------
                from concourse import bass_isa

                s_all = smp.tile([128, IC], f32, tag="sall")
                nc.gpsimd.partition_all_reduce(
                    s_all, colsum, 128, bass_isa.ReduceOp.add
                )
                sinv_tab = smp.tile([128, IC], f32, tag="stab")
                nc.vector.reciprocal(out=sinv_tab, in_=s_all)

                # ---------- s phase + squash, one half at a time ----------
                si_tiles = []
                for i in range(IC):
                    si = sip.tile([128, 128], bf16, tag="si", name=f"si{i}")
                    nc.gpsimd.tensor_scalar(
                        out=si,
                        in0=i128s,
                        scalar1=sinv_tab[:, i : i + 1],
                        scalar2=float(L),
                        op0=OP.mult,
                        op1=OP.mult,
                    )
                    si_tiles.append(si)
                o_t = sqp.tile([128, 2, HP], f32, tag="ot")
                for h in range(2):
                    psum_s = ltp.tile([128, 512], f32, tag="lt")
                    for i in range(IC):
                        nc.tensor.matmul(
                            psum_s[:, 0:HP],
                            si_tiles[i],
                            q2_tiles[i][:, h, :],
                            start=(i == 0),
                            stop=(i == IC - 1),
                        )
                    v_sb = sqp.tile([128, HP], f32, tag="vsb")
                    nc.vector.tensor_add(
                        out=v_sb, in0=psum_s[:, 0:HP], in1=t2_f[:, h, :]
                    )
                    sq_bf = sqp.tile([128, HP], bf16, tag="sqbf")
                    nc.scalar.activation(out=sq_bf, in_=v_sb, func=AF.Square)
                    pn = ltp.tile([128, 512], f32, tag="lt")
                    nc.tensor.matmul(
                        pn[:, 0:HP], lreps, sq_bf, start=True, stop=True
                    )
                    rsb = sqp.tile([128, HP], f32, tag="rsb")
                    nc.scalar.activation(
                        out=rsb, in_=pn[:, 0:HP], func=AF.Sqrt, bias=eps_t
                    )
                    rinv = sqp.tile([128, HP], f32, tag="rinv")
                    nc.vector.reciprocal(out=rinv, in_=rsb)
                    g_t = sqp.tile([128, HP], f32, tag="gt")
                    nc.scalar.activation(out=g_t, in_=rsb, func=AF.Exp, scale=-1.0)
                    nc.vector.tensor_scalar(
                        out=g_t,
                        in0=g_t,
                        scalar1=-1.0,
                        scalar2=1.0,
                        op0=OP.mult,
                        op1=OP.add,
                    )
                    a_t = sqp.tile([128, HP], f32, tag="at")
                    nc.vector.tensor_mul(out=a_t, in0=v_sb, in1=rinv)
                    nc.vector.tensor_mul(out=o_t[:, h, :], in0=a_t, in1=g_t)
                nc.gpsimd.dma_start(
                    out=y_d[b].rearrange("p (h n) -> p h n", h=2), in_=o_t
                )

    nc.finalize()
    return nc


def _prep_host(x, w, b_route):
    import ml_dtypes

    bf = ml_dtypes.bfloat16
    x = np.ascontiguousarray(np.asarray(x, dtype=np.float32))
    w = np.asarray(w, dtype=np.float32)
    b_route = np.asarray(b_route, dtype=np.float32)

    # xt[b, i, il, hw]
    xt = np.ascontiguousarray(x.transpose(0, 3, 4, 1, 2)).reshape(B, IC, IL, HW)
    xt9 = np.zeros((B, IC, K9, HW), dtype=bf)
    xtb = xt.astype(bf)
    for g, s in enumerate(SHIFTS):
        if s == 0:
            xt9[:, :, g * IL : (g + 1) * IL, :] = xtb
        else:
            xt9[:, :, g * IL : (g + 1) * IL, : HW - s] = xtb[:, :, :, s:]

    xnat = x.reshape(B, HW, IC * IL)

    # W72[(ky,kx,il), cl]
    w72 = np.ascontiguousarray(
        w[:, :, :, 0, :].transpose(1, 2, 0, 3).reshape(K9, CL)
    ).astype(bf)
    lrep = np.kron(np.eye(C, dtype=np.float32), np.ones((L, L), np.float32)).astype(bf)
    i128 = np.eye(128, dtype=np.float32).astype(bf)
    # br_cl[(c*8+l), pos] = b_route[pos*16+c, l]
    br_cl = np.ascontiguousarray(
        b_route.reshape(POS, C, L).transpose(1, 2, 0).reshape(128, POS)
    ).astype(np.float32)
    return xt9, xnat, w72, lrep, i128, br_cl


def kernel(x, w, b_route, stride):
    assert int(stride) == 1
    xt9, xnat, w72, lrep, i128, br_cl = _prep_host(x, w, b_route)

    if "nc" not in _CACHE:
        _CACHE["nc"] = _build_nc()
    nc = _CACHE["nc"]

    from concourse.bass_utils import run_bass_kernel_spmd

    in_maps = []
    for c in range(NCORES):
        sl = slice(c * BLOC, (c + 1) * BLOC)
        in_maps.append(
            {
                "xt9": np.ascontiguousarray(xt9[sl]),
                "xnat": np.ascontiguousarray(xnat[sl]),
                "w72": w72,
                "lrep": lrep,
                "i128": i128,
                "br_cl": br_cl,
            }
        )

    res = run_bass_kernel_spmd(nc, in_maps, core_ids=list(range(NCORES)))

    y = np.empty((B, OH, OW, C, L), dtype=np.float32)
    for c in range(NCORES):
        yd = res.results[c]["y"]  # [BLOC, 128, 900]
        y[c * BLOC : (c + 1) * BLOC] = (
            yd.reshape(BLOC, C, L, POS).transpose(0, 3, 1, 2).reshape(
                BLOC, OH, OW, C, L
            )
        )
    return y


# revision 31
# speedup vs baseline: 7266.3151x; 1.0052x over previous
"""Trainium2 Bass/Tile kernel for nn_Capsule3D (capsule conv + routing softmax + squash).

Sharding: data-parallel over batch, 2 samples per core x 8 cores. Host side does
only layout transforms (transpose / 9-shift im2col row replication / dtype casts)
and sharding; all math runs on the NeuronCores.

Per sample b, on device (layout: partitions = (c,l) = 128 output channels,
free = output positions pos = 900, per input capsule i = 0..31):
  - t = conv(sum_i x_i) via a mini 72x128 matmul (conv is linear in its input,
    so the routing sum over capsules commutes with the conv)
  - main loop per i: K=72 weights-stationary conv matmul -> PSUM; evict to bf16
    (split ScalarE/VectorE); q = u_hat*t (VectorE bf16 2x); "Lrep" matmul with a
    block-diagonal ones matrix reduces over l AND replicates the result over the
    l partitions; exp((z)/sqrt L) on ScalarE straight from PSUM with accum_out
    giving the softmax denominator column-sums for free; q2 = u_hat*e (VectorE).
  - softmax denominators via gpsimd partition_all_reduce; 1/S_i folded into
    per-i scaled-identity matmuls ("si") built on GpSimd.
  - s = sum_i si^T @ q2_i accumulated in PSUM by TensorE (+ t*b_route term),
    so the s-phase is matmul-only and overlaps the next sample's main loop.
  - squash: norm over l via Lrep matmul on v^2, then v*(1-exp(-r))/r.

The softmax skips the max-subtraction (logits are O(5), safe in fp32 exp).
Intermediates are bf16 (measured end-to-end error ~9e-3 scale-relative absmax
vs the fp32 reference, i.e. ~1e-2 l2-relative, under the 2e-2 gate).
"""

import math

import numpy as np

# ---------------- problem constants (hardcoded per harness contract) ----------
B, H, W, IC, IL = 16, 32, 32, 32, 8
KH = KW = 3
CL = 128
L = 8
C = CL // L            # 16
OH = OW = 30
POS = OH * OW          # 900
HW = H * W             # 1024
K9 = KH * KW * IL      # 72
NCORES = 8
BLOC = B // NCORES     # 2
EPS = 1e-7
RSQRT_L = 1.0 / math.sqrt(float(L))
SHIFTS = [32 * ky + kx for ky in range(KH) for kx in range(KW)]

_CACHE = {}


def _build_nc():
    import concourse.tile as tile
    from concourse import bacc, mybir

    f32 = mybir.dt.float32
    bf16 = mybir.dt.bfloat16
    AF = mybir.ActivationFunctionType
    OP = mybir.AluOpType

    nc = bacc.Bacc()

    xt9_d = nc.dram_tensor("xt9", [BLOC, IC, K9, HW], bf16, kind="ExternalInput")
    xnat_d = nc.dram_tensor("xnat", [BLOC, HW, IC * IL], f32, kind="ExternalInput")
    w72_d = nc.dram_tensor("w72", [K9, CL], bf16, kind="ExternalInput")
    lrep_d = nc.dram_tensor("lrep", [128, 128], bf16, kind="ExternalInput")
    i128_d = nc.dram_tensor("i128", [128, 128], bf16, kind="ExternalInput")
    br_d = nc.dram_tensor("br_cl", [128, POS], f32, kind="ExternalInput")
    y_d = nc.dram_tensor("y", [BLOC, 128, POS], f32, kind="ExternalOutput")

    HP = 450  # half of the 900 output positions

    with tile.TileContext(nc) as tc:
        with (
            tc.tile_pool(name="const", bufs=1) as constp,
            tc.tile_pool(name="xnat", bufs=2) as xnatp,
            tc.tile_pool(name="ub", bufs=4) as ubp,
            tc.tile_pool(name="ubar", bufs=1) as ubarp,
            tc.tile_pool(name="xt9", bufs=4) as xt9p,
            tc.tile_pool(name="utmp", bufs=5) as utmpp,
            tc.tile_pool(name="etmp", bufs=5) as etmpp,
            tc.tile_pool(name="q2s", bufs=IC) as q2p,
            tc.tile_pool(name="tt", bufs=2) as ttp,
            tc.tile_pool(name="q", bufs=8) as qp,
            tc.tile_pool(name="sip", bufs=IC) as sip,
            tc.tile_pool(name="sm", bufs=2) as smp,
            tc.tile_pool(name="sq", bufs=3) as sqp,
            tc.tile_pool(name="pu", bufs=3, space="PSUM") as pup,
            tc.tile_pool(name="pz", bufs=2, space="PSUM") as pzp,
            tc.tile_pool(name="lt", bufs=1, space="PSUM") as ltp,
        ):
            # ---- constants (loaded once) ----
            w72s = constp.tile([K9, CL], bf16)
            nc.sync.dma_start(out=w72s, in_=w72_d[:, :])
            lreps = constp.tile([128, 128], bf16)
            nc.sync.dma_start(out=lreps, in_=lrep_d[:, :])
            i128s = constp.tile([128, 128], bf16)
            nc.sync.dma_start(out=i128s, in_=i128_d[:, :])
            brs = constp.tile([128, POS], f32)
            nc.sync.dma_start(out=brs, in_=br_d[:, :])
            eps_t = constp.tile([128, 1], f32)
            nc.vector.memset(eps_t, EPS)

            for b in range(BLOC):
                # ---------- ubar path ----------
                ubarT = ubarp.tile([IL, HW], bf16, tag="ubarT")
                xn = xnatp.tile([128, HW // 128, IC * IL], f32, tag="xn")
                nc.sync.dma_start(
                    out=xn, in_=xnat_d[b].rearrange("(t p) f -> p t f", p=128)
                )
                for hwt in range(HW // 128):
                    ub_f = ubp.tile([128, IL], f32, tag="ubf")
                    nc.vector.reduce_sum(
                        out=ub_f,
                        in_=xn[:, hwt, :].rearrange("p (i l) -> p l i", l=IL),
                        axis=mybir.AxisListType.X,
                    )
                    ub_b = ubp.tile([128, IL], bf16, tag="ubb")
                    nc.vector.tensor_copy(out=ub_b, in_=ub_f)
                    ps_tr = pup.tile([IL, 128], bf16, tag="pu")
                    nc.tensor.transpose(ps_tr, ub_b, i128s)
                    nc.vector.tensor_copy(
                        out=ubarT[:, hwt * 128 : (hwt + 1) * 128], in_=ps_tr
                    )
                ubar9 = ubarp.tile([K9, HW], bf16, tag="ubar9")
                for g, s in enumerate(SHIFTS):
                    nc.sync.dma_start(
                        out=ubar9[g * IL : (g + 1) * IL, 0 : HW - s],
                        in_=ubarT[:, s:HW],
                    )

                # ---------- t mini-conv (per half) ----------
                ub_v = ubar9.rearrange("p (h w) -> p h w", w=W)
                t_bf = ttp.tile([128, 2, HP], bf16, tag="tbf")
                t2_f = ttp.tile([128, 2, HP], f32, tag="t2")
                brv = brs.rearrange("p (h n) -> p h n", h=2)
                for h in range(2):
                    psum_t = pup.tile([128, 512], f32, tag="pu")
                    nc.tensor.matmul(
                        psum_t[:, 0:HP],
                        w72s,
                        ub_v[:, 15 * h : 15 * h + 15, 0:OW],
                        start=True,
                        stop=True,
                    )
                    nc.scalar.copy(out=t_bf[:, h, :], in_=psum_t[:, 0:HP])
                    nc.vector.tensor_mul(
                        out=t2_f[:, h, :], in0=psum_t[:, 0:HP], in1=brv[:, h, :]
                    )

                # ---------- main loop: conv, evict, z, l-reduce, exp, q2 ----------
                q2_tiles = []
                colsum = smp.tile([128, IC], f32, tag="colsum")
                for i in range(IC):
                    xt9 = xt9p.tile([K9, HW], bf16, tag="xt9")
                    nc.sync.dma_start(out=xt9, in_=xt9_d[b, i])
                    xv = xt9.rearrange("p (h w) -> p h w", w=W)
                    U_i = utmpp.tile([128, 2, HP], bf16, tag="ut")
                    e_i = etmpp.tile([128, 2, HP], bf16, tag="et")
                    pz = pzp.tile([128, 2, 512], f32, tag="pz")
                    for h in range(2):
                        pu = pup.tile([128, 512], f32, tag="pu")
                        nc.tensor.matmul(
                            pu[:, 0:HP],
                            w72s,
                            xv[:, 15 * h : 15 * h + 15, 0:OW],
                            start=True,
                            stop=True,
                        )
                        if (2 * i + h) % 2 == 0:
                            nc.vector.tensor_copy(out=U_i[:, h, :], in_=pu[:, 0:HP])
                        else:
                            nc.scalar.copy(out=U_i[:, h, :], in_=pu[:, 0:HP])
                    q = qp.tile([128, 2, HP], bf16, tag="q")
                    nc.vector.tensor_mul(out=q, in0=U_i, in1=t_bf)
                    for h in range(2):
                        nc.tensor.matmul(
                            pz[:, h, 0:HP], lreps, q[:, h, :], start=True, stop=True
                        )
                    nc.scalar.activation(
                        out=e_i,
                        in_=pz[:, :, 0:HP],
                        func=AF.Exp,
                        scale=RSQRT_L,
                        accum_out=colsum[:, i : i + 1],
                    )
                    q2_i = q2p.tile([128, 2, HP], bf16, tag="q2", name=f"q2_{i}")
                    q2_tiles.append(q2_i)
                    nc.vector.tensor_mul(out=q2_i, in0=U_i, in1=e_i)

                # ---------- softmax denominator: allreduce over partitions ----------
                from concourse import bass_isa

                s_all = smp.tile([128, IC], f32, tag="sall")
                nc.gpsimd.partition_all_reduce(
                    s_all, colsum, 128, bass_isa.ReduceOp.add
                )
                sinv_tab = smp.tile([128, IC], f32, tag="stab")
                nc.vector.reciprocal(out=sinv_tab, in_=s_all)

                # ---------- s phase + squash, one half at a time ----------
                si_tiles = []
                for i in range(IC):
                    si = sip.tile([128, 128], bf16, tag="si", name=f"si{i}")
                    nc.gpsimd.tensor_scalar(
                        out=si,
                        in0=i128s,
                        scalar1=sinv_tab[:, i : i + 1],
                        scalar2=float(L),
                        op0=OP.mult,
                        op1=OP.mult,
                    )
                    si_tiles.append(si)
                o_t = sqp.tile([128, 2, HP], f32, tag="ot")
                for h in range(2):
                    psum_s = ltp.tile([128, 512], f32, tag="lt")
                    for i in range(IC):
                        nc.tensor.matmul(
                            psum_s[:, 0:HP],
                            si_tiles[i],
                            q2_tiles[i][:, h, :],
                            start=(i == 0),
                            stop=(i == IC - 1),
                        )
                    v_sb = sqp.tile([128, HP], f32, tag="vsb")
                    nc.vector.tensor_add(
                        out=v_sb, in0=psum_s[:, 0:HP], in1=t2_f[:, h, :]
                    )
                    sq_bf = sqp.tile([128, HP], bf16, tag="sqbf")
                    nc.scalar.activation(out=sq_bf, in_=v_sb, func=AF.Square)
                    pn = ltp.tile([128, 512], f32, tag="lt")
                    nc.tensor.matmul(
                        pn[:, 0:HP], lreps, sq_bf, start=True, stop=True
                    )
                    rsb = sqp.tile([128, HP], f32, tag="rsb")
                    nc.scalar.activation(
                        out=rsb, in_=pn[:, 0:HP], func=AF.Sqrt, bias=eps_t
                    )
                    rinv = sqp.tile([128, HP], f32, tag="rinv")
                    nc.vector.reciprocal(out=rinv, in_=rsb)
                    g_t = sqp.tile([128, HP], f32, tag="gt")
                    nc.scalar.activation(out=g_t, in_=rsb, func=AF.Exp, scale=-1.0)
                    nc.vector.tensor_scalar(
                        out=g_t,
                        in0=g_t,
                        scalar1=-1.0,
                        scalar2=1.0,
                        op0=OP.mult,
                        op1=OP.add,
                    )
                    a_t = sqp.tile([128, HP], f32, tag="at")
                    nc.vector.tensor_mul(out=a_t, in0=v_sb, in1=rinv)
                    nc.vector.tensor_mul(out=o_t[:, h, :], in0=a_t, in1=g_t)
                nc.gpsimd.dma_start(
                    out=y_d[b].rearrange("p (h n) -> p h n", h=2), in_=o_t
                )

    nc.finalize()
    return nc


def _prep_host(x, w, b_route):
    import ml_dtypes

    bf = ml_dtypes.bfloat16
    x = np.ascontiguousarray(np.asarray(x, dtype=np.float32))
    w = np.asarray(w, dtype=np.float32)
    b_route = np.asarray(b_route, dtype=np.float32)

    # xt[b, i, il, hw]
    xt = np.ascontiguousarray(x.transpose(0, 3, 4, 1, 2)).reshape(B, IC, IL, HW)
    xt9 = np.zeros((B, IC, K9, HW), dtype=bf)
    xtb = xt.astype(bf)
    for g, s in enumerate(SHIFTS):
        if s == 0:
            xt9[:, :, g * IL : (g + 1) * IL, :] = xtb
        else:
            xt9[:, :, g * IL : (g + 1) * IL, : HW - s] = xtb[:, :, :, s:]

    xnat = x.reshape(B, HW, IC * IL)

    # W72[(ky,kx,il), cl]
    w72 = np.ascontiguousarray(
        w[:, :, :, 0, :].transpose(1, 2, 0, 3).reshape(K9, CL)
    ).astype(bf)
    lrep = np.kron(np.eye(C, dtype=np.float32), np.ones((L, L), np.float32)).astype(bf)
    i128 = np.eye(128, dtype=np.float32).astype(bf)
    # br_cl[(c*8+l), pos] = b_route[pos*16+c, l]
    br_cl = np.ascontiguousarray(
        b_route.reshape(POS, C, L).transpose(1, 2, 0).reshape(128, POS)
    ).astype(np.float32)
    return xt9, xnat, w72, lrep, i128, br_cl


def kernel(x, w, b_route, stride):
    assert int(stride) == 1
    xt9, xnat, w72, lrep, i128, br_cl = _prep_host(x, w, b_route)

    if "nc" not in _CACHE:
        _CACHE["nc"] = _build_nc()
    nc = _CACHE["nc"]

    from concourse.bass_utils import run_bass_kernel_spmd

    in_maps = []
    for c in range(NCORES):
        sl = slice(c * BLOC, (c + 1) * BLOC)
        in_maps.append(
            {
                "xt9": np.ascontiguousarray(xt9[sl]),
                "xnat": np.ascontiguousarray(xnat[sl]),
                "w72": w72,
                "lrep": lrep,
                "i128": i128,
                "br_cl": br_cl,
            }
        )

    res = run_bass_kernel_spmd(nc, in_maps, core_ids=list(range(NCORES)))

    y = np.empty((B, OH, OW, C, L), dtype=np.float32)
    for c in range(NCORES):
        yd = res.results[c]["y"]  # [BLOC, 128, 900]
        y[c * BLOC : (c + 1) * BLOC] = (
            yd.reshape(BLOC, C, L, POS).transpose(0, 3, 1, 2).reshape(
                BLOC, OH, OW, C, L
            )
        )
    return y


# revision 36
# speedup vs baseline: 7339.4341x; 1.0101x over previous
"""Trainium2 Bass/Tile kernel for nn_Capsule3D (capsule conv + routing softmax + squash).

Sharding: data-parallel over batch, 2 samples per core x 8 cores. Host side does
only layout transforms (transpose / 9-shift im2col row replication / dtype casts)
and sharding; all math runs on the NeuronCores.

Per sample b, on device (layout: partitions = (c,l) = 128 output channels,
free = output positions pos = 900, per input capsule i = 0..31):
  - t = conv(sum_i x_i) via a mini 72x128 matmul (conv is linear in its input,
    so the routing sum over capsules commutes with the conv)
  - main loop per i: K=72 weights-stationary conv matmul -> PSUM; evict to bf16
    (split ScalarE/VectorE); q = u_hat*t (VectorE bf16 2x); "Lrep" matmul with a
    block-diagonal ones matrix reduces over l AND replicates the result over the
    l partitions; exp((z)/sqrt L) on ScalarE straight from PSUM with accum_out
    giving the softmax denominator column-sums for free; q2 = u_hat*e (VectorE).
  - softmax denominators via gpsimd partition_all_reduce; 1/S_i folded into
    per-i scaled-identity matmuls ("si") built on GpSimd.
  - s = sum_i si^T @ q2_i accumulated in PSUM by TensorE (+ t*b_route term),
    so the s-phase is matmul-only and overlaps the next sample's main loop.
  - squash: norm over l via Lrep matmul on v^2, then v*(1-exp(-r))/r.

The softmax skips the max-subtraction (logits are O(5), safe in fp32 exp).
Intermediates are bf16 (measured end-to-end error ~9e-3 scale-relative absmax
vs the fp32 reference, i.e. ~1e-2 l2-relative, under the 2e-2 gate).
"""

import math

import numpy as np

# ---------------- problem constants (hardcoded per harness contract) ----------
B, H, W, IC, IL = 16, 32, 32, 32, 8
KH = KW = 3
CL = 128
L = 8
C = CL // L            # 16
OH = OW = 30
POS = OH * OW          # 900
HW = H * W             # 1024
K9 = KH * KW * IL      # 72
NCORES = 8
BLOC = B // NCORES     # 2
EPS = 1e-7
RSQRT_L = 1.0 / math.sqrt(float(L))
SHIFTS = [32 * ky + kx for ky in range(KH) for kx in range(KW)]

_CACHE = {}


def _build_nc():
    import concourse.tile as tile
    from concourse import bacc, mybir

    f32 = mybir.dt.float32
    bf16 = mybir.dt.bfloat16
    AF = mybir.ActivationFunctionType
    OP = mybir.AluOpType

    nc = bacc.Bacc()

    xt9_d = nc.dram_tensor("xt9", [BLOC, IC, K9, HW], bf16, kind="ExternalInput")
    xnat_d = nc.dram_tensor("xnat", [BLOC, HW, IC * IL], f32, kind="ExternalInput")
    w72_d = nc.dram_tensor("w72", [K9, CL], bf16, kind="ExternalInput")
    lrep_d = nc.dram_tensor("lrep", [128, 128], bf16, kind="ExternalInput")
    i128_d = nc.dram_tensor("i128", [128, 128], bf16, kind="ExternalInput")
    br_d = nc.dram_tensor("br_cl", [128, POS], f32, kind="ExternalInput")
    y_d = nc.dram_tensor("y", [BLOC, 128, POS], f32, kind="ExternalOutput")

    HP = 450  # half of the 900 output positions

    with tile.TileContext(nc) as tc:
        with (
            tc.tile_pool(name="const", bufs=1) as constp,
            tc.tile_pool(name="xnat", bufs=2) as xnatp,
            tc.tile_pool(name="ub", bufs=4) as ubp,
            tc.tile_pool(name="ubar", bufs=1) as ubarp,
            tc.tile_pool(name="xt9", bufs=4) as xt9p,
            tc.tile_pool(name="utmp", bufs=5) as utmpp,
            tc.tile_pool(name="etmp", bufs=5) as etmpp,
            tc.tile_pool(name="q2s", bufs=IC) as q2p,
            tc.tile_pool(name="tt", bufs=2) as ttp,
            tc.tile_pool(name="q", bufs=8) as qp,
            tc.tile_pool(name="sip", bufs=IC) as sip,
            tc.tile_pool(name="sm", bufs=2) as smp,
            tc.tile_pool(name="sq", bufs=3) as sqp,
            tc.tile_pool(name="pu", bufs=3, space="PSUM") as pup,
            tc.tile_pool(name="pz", bufs=2, space="PSUM") as pzp,
            tc.tile_pool(name="lt", bufs=1, space="PSUM") as ltp,
        ):
            # ---- constants (loaded once) ----
            w72s = constp.tile([K9, CL], bf16)
            nc.sync.dma_start(out=w72s, in_=w72_d[:, :])
            lreps = constp.tile([128, 128], bf16)
            nc.sync.dma_start(out=lreps, in_=lrep_d[:, :])
            i128s = constp.tile([128, 128], bf16)
            nc.sync.dma_start(out=i128s, in_=i128_d[:, :])
            brs = constp.tile([128, POS], f32)
            nc.gpsimd.dma_start(out=brs, in_=br_d[:, :])
            eps_t = constp.tile([128, 1], f32)
            nc.vector.memset(eps_t, EPS)

            for b in range(BLOC):
                # ---------- ubar path ----------
                ubarT = ubarp.tile([IL, HW], bf16, tag="ubarT")
                xn = xnatp.tile([128, HW // 128, IC * IL], f32, tag="xn")
                nc.sync.dma_start(
                    out=xn, in_=xnat_d[b].rearrange("(t p) f -> p t f", p=128)
                )
                for hwt in range(HW // 128):
                    ub_f = ubp.tile([128, IL], f32, tag="ubf")
                    nc.vector.reduce_sum(
                        out=ub_f,
                        in_=xn[:, hwt, :].rearrange("p (i l) -> p l i", l=IL),
                        axis=mybir.AxisListType.X,
                    )
                    ub_b = ubp.tile([128, IL], bf16, tag="ubb")
                    nc.scalar.copy(out=ub_b, in_=ub_f)
                    ps_tr = pup.tile([IL, 128], bf16, tag="pu")
                    nc.tensor.transpose(ps_tr, ub_b, i128s)
                    nc.vector.tensor_copy(
                        out=ubarT[:, hwt * 128 : (hwt + 1) * 128], in_=ps_tr
                    )
                ubar9 = ubarp.tile([K9, HW], bf16, tag="ubar9")
                for g, s in enumerate(SHIFTS):
                    nc.sync.dma_start(
                        out=ubar9[g * IL : (g + 1) * IL, 0 : HW - s],
                        in_=ubarT[:, s:HW],
                    )

                # ---------- t mini-conv (per half) ----------
                ub_v = ubar9.rearrange("p (h w) -> p h w", w=W)
                t_bf = ttp.tile([128, 2, HP], bf16, tag="tbf")
                t2_f = ttp.tile([128, 2, HP], f32, tag="t2")
                brv = brs.rearrange("p (h n) -> p h n", h=2)
                for h in range(2):
                    psum_t = pup.tile([128, 512], f32, tag="pu")
                    nc.tensor.matmul(
                        psum_t[:, 0:HP],
                        w72s,
                        ub_v[:, 15 * h : 15 * h + 15, 0:OW],
                        start=True,
                        stop=True,
                    )
                    nc.scalar.copy(out=t_bf[:, h, :], in_=psum_t[:, 0:HP])
                    nc.vector.tensor_mul(
                        out=t2_f[:, h, :], in0=psum_t[:, 0:HP], in1=brv[:, h, :]
                    )

                # ---------- main loop: conv, evict, z, l-reduce, exp, q2 ----------
                q2_tiles = []
                colsum = smp.tile([128, IC], f32, tag="colsum")
                for i in range(IC):
                    xt9 = xt9p.tile([K9, HW], bf16, tag="xt9")
                    nc.sync.dma_start(out=xt9, in_=xt9_d[b, i])
                    xv = xt9.rearrange("p (h w) -> p h w", w=W)
                    U_i = utmpp.tile([128, 2, HP], bf16, tag="ut")
                    e_i = etmpp.tile([128, 2, HP], bf16, tag="et")
                    pz = pzp.tile([128, 2, 512], f32, tag="pz")
                    for h in range(2):
                        pu = pup.tile([128, 512], f32, tag="pu")
                        nc.tensor.matmul(
                            pu[:, 0:HP],
                            w72s,
                            xv[:, 15 * h : 15 * h + 15, 0:OW],
                            start=True,
                            stop=True,
                        )
                        if (2 * i + h) % 2 == 0:
                            nc.vector.tensor_copy(out=U_i[:, h, :], in_=pu[:, 0:HP])
                        else:
                            nc.scalar.copy(out=U_i[:, h, :], in_=pu[:, 0:HP])
                    q = qp.tile([128, 2, HP], bf16, tag="q")
                    nc.vector.tensor_mul(out=q, in0=U_i, in1=t_bf)
                    for h in range(2):
                        nc.tensor.matmul(
                            pz[:, h, 0:HP], lreps, q[:, h, :], start=True, stop=True
                        )
                    nc.scalar.activation(
                        out=e_i,
                        in_=pz[:, :, 0:HP],
                        func=AF.Exp,
                        scale=RSQRT_L,
                        accum_out=colsum[:, i : i + 1],
                    )
                    q2_i = q2p.tile([128, 2, HP], bf16, tag="q2", name=f"q2_{i}")
                    q2_tiles.append(q2_i)
                    nc.vector.tensor_mul(out=q2_i, in0=U_i, in1=e_i)

                # ---------- softmax denominator: allreduce over partitions ----------
                from concourse import bass_isa

                s_all = smp.tile([128, IC], f32, tag="sall")
                nc.gpsimd.partition_all_reduce(
                    s_all, colsum, 128, bass_isa.ReduceOp.add
                )
                sinv_tab = smp.tile([128, IC], f32, tag="stab")
                nc.vector.reciprocal(out=sinv_tab, in_=s_all)

                # ---------- s phase + squash, one half at a time ----------
                si_tiles = []
                si_eng = nc.gpsimd if b == 0 else nc.vector
                for i in range(IC):
                    si = sip.tile([128, 128], bf16, tag="si", name=f"si{i}")
                    si_eng.tensor_scalar(
                        out=si,
                        in0=i128s,
                        scalar1=sinv_tab[:, i : i + 1],
                        scalar2=float(L),
                        op0=OP.mult,
                        op1=OP.mult,
                    )
                    si_tiles.append(si)
                o_t = sqp.tile([128, 2, HP], f32, tag="ot")
                for h in range(2):
                    psum_s = ltp.tile([128, 512], f32, tag="lt")
                    for i in range(IC):
                        nc.tensor.matmul(
                            psum_s[:, 0:HP],
                            si_tiles[i],
                            q2_tiles[i][:, h, :],
                            start=(i == 0),
                            stop=(i == IC - 1),
                        )
                    v_sb = sqp.tile([128, HP], f32, tag="vsb")
                    nc.vector.tensor_add(
                        out=v_sb, in0=psum_s[:, 0:HP], in1=t2_f[:, h, :]
                    )
                    sq_bf = sqp.tile([128, HP], bf16, tag="sqbf")
                    nc.scalar.activation(out=sq_bf, in_=v_sb, func=AF.Square)
                    pn = ltp.tile([128, 512], f32, tag="lt")
                    nc.tensor.matmul(
                        pn[:, 0:HP], lreps, sq_bf, start=True, stop=True
                    )
                    rsb = sqp.tile([128, HP], f32, tag="rsb")
                    nc.scalar.activation(
                        out=rsb, in_=pn[:, 0:HP], func=AF.Sqrt, bias=eps_t
                    )
                    rinv = sqp.tile([128, HP], f32, tag="rinv")
                    nc.vector.reciprocal(out=rinv, in_=rsb)
                    g_t = sqp.tile([128, HP], f32, tag="gt")
                    nc.scalar.activation(out=g_t, in_=rsb, func=AF.Exp, scale=-1.0)
                    nc.vector.tensor_scalar(
                        out=g_t,
                        in0=g_t,
                        scalar1=-1.0,
                        scalar2=1.0,
                        op0=OP.mult,
                        op1=OP.add,
                    )
                    a_t = sqp.tile([128, HP], f32, tag="at")
                    nc.vector.tensor_mul(out=a_t, in0=v_sb, in1=rinv)
                    nc.vector.tensor_mul(out=o_t[:, h, :], in0=a_t, in1=g_t)
                nc.gpsimd.dma_start(
                    out=y_d[b].rearrange("p (h n) -> p h n", h=2), in_=o_t
                )

    nc.finalize()
    return nc


def _prep_host(x, w, b_route):
    import ml_dtypes

    bf = ml_dtypes.bfloat16
    x = np.ascontiguousarray(np.asarray(x, dtype=np.float32))
    w = np.asarray(w, dtype=np.float32)
    b_route = np.asarray(b_route, dtype=np.float32)

    # xt[b, i, il, hw]
    xt = np.ascontiguousarray(x.transpose(0, 3, 4, 1, 2)).reshape(B, IC, IL, HW)
    xt9 = np.zeros((B, IC, K9, HW), dtype=bf)
    xtb = xt.astype(bf)
    for g, s in enumerate(SHIFTS):
        if s == 0:
            xt9[:, :, g * IL : (g + 1) * IL, :] = xtb
        else:
            xt9[:, :, g * IL : (g + 1) * IL, : HW - s] = xtb[:, :, :, s:]

    xnat = x.reshape(B, HW, IC * IL)

    # W72[(ky,kx,il), cl]
    w72 = np.ascontiguousarray(
        w[:, :, :, 0, :].transpose(1, 2, 0, 3).reshape(K9, CL)
    ).astype(bf)
    lrep = np.kron(np.eye(C, dtype=np.float32), np.ones((L, L), np.float32)).astype(bf)
    i128 = np.eye(128, dtype=np.float32).astype(bf)
    # br_cl[(c*8+l), pos] = b_route[pos*16+c, l]
    br_cl = np.ascontiguousarray(
        b_route.reshape(POS, C, L).transpose(1, 2, 0).reshape(128, POS)
    ).astype(np.float32)
    return xt9, xnat, w72, lrep, i128, br_cl


def kernel(x, w, b_route, stride):
    assert int(stride) == 1
    xt9, xnat, w72, lrep, i128, br_cl = _prep_host(x, w, b_route)

    if "nc" not in _CACHE:
        _CACHE["nc"] = _build_nc()
    nc = _CACHE["nc"]

    from concourse.bass_utils import run_bass_kernel_spmd

    in_maps = []
    for c in range(NCORES):
        sl = slice(c * BLOC, (c + 1) * BLOC)
        in_maps.append(
            {
                "xt9": np.ascontiguousarray(xt9[sl]),
                "xnat": np.ascontiguousarray(xnat[sl]),
                "w72": w72,
                "lrep": lrep,
                "i128": i128,
                "br_cl": br_cl,
            }
        )

    res = run_bass_kernel_spmd(nc, in_maps, core_ids=list(range(NCORES)))

    y = np.empty((B, OH, OW, C, L), dtype=np.float32)
    for c in range(NCORES):
        yd = res.results[c]["y"]  # [BLOC, 128, 900]
        y[c * BLOC : (c + 1) * BLOC] = (
            yd.reshape(BLOC, C, L, POS).transpose(0, 3, 1, 2).reshape(
                BLOC, OH, OW, C, L
            )
        )
    return y
